# revision 22
# baseline (speedup 1.0000x reference)
"""AdditiveAttention (Bahdanau) distributed Bass kernel for 8 TRN2 NeuronCores.

Computation (per batch b):
    qc[b,:]   = query[b] @ Wq + bq + bv                       # [512]
    z[b,s,:]  = value[b,s] @ Wv + qc[b]                       # pre-tanh
    score     = tanh(z) @ Wo          (+bo dropped: cancels in softmax)
    align     = softmax(score)        (no max-sub: |score| <= ~23, exp fits f32)
    out[b,:]  = align @ value[b]

Sharding: data-parallel over batch, 4 batches per core, weights replicated.

Per-core dataflow (B=4 batches, SEQ=4096, H=512), hidden-TRANSPOSED layout:
  - value loaded HBM->SBUF with f32->bf16 cast DMA (SWDGE), natural layout
    v_nat[128s, 4t, 512h] per 512-seq block.
  - xbar DMA-transpose (HWDGE) per s-tile: [128s,512h] -> vT[128p,4k,128s]
    with h = 128k + p (verified on HW).
  - main mm per (blk, hoc): psum_hT[128ho, 512s] = sum_k Wv[:,k,hoc].T @ vT[:,k,:]
    (Wv chunk stationary, vT moving, N=512).
  - tanh on ACT with per-partition bias qcombT[ho] -> hT bf16 (query proj free).
  - score per blk: psum[1,512] = sum_hoc Wo[:,hoc].T @ hT[:,hoc,:]; DVE copy
    into scrow[1,4096].
  - one Exp per batch on ACT [1,4096] with accum_out -> esc_row bf16 + total;
    DVE reciprocal.
  - escT via 32 tiny PE transpose-matmuls (esc row chunk [1,128] stationary,
    ones rhs) -> psum [128,1] -> DVE copy -> escT[128,32] bf16.
  - context: psum[1,512] += escT[:,t].T @ v_nat (32 accumulating mms);
    DVE tensor_scalar_mul by 1/total -> out row.
  - batch tail (escT+ctx+out) is emitted 2 blocks into the NEXT batch so the
    PE stream never drains at batch boundaries (HAM stays warm).
"""

import numpy as np

N_CORES = 8
BATCH_TOTAL = 32
B = BATCH_TOTAL // N_CORES  # batches per core
SEQ = 4096
H = 512
HC = H // 128  # 4 hidden chunks

_cache = {}


def build_nc(b_per_core=B, seq=SEQ):
    import concourse.bass as bass
    import concourse.mybir as mybir
    import concourse.tile as tile
    from concourse import bacc
    from concourse.masks import make_identity
    from concourse.bass_isa import ReduceOp

    f32 = mybir.dt.float32
    bf16 = mybir.dt.bfloat16
    AF = mybir.ActivationFunctionType
    ALU = mybir.AluOpType
    AX = mybir.AxisListType

    nblk = seq // 512   # 512-seq blocks
    nt = seq // 128     # 128-seq tiles

    nc = bacc.Bacc("TRN2", target_bir_lowering=False, debug=False)

    val_d = nc.dram_tensor("value", [b_per_core, seq, H], f32, kind="ExternalInput").ap()
    q_d = nc.dram_tensor("query", [b_per_core, H], f32, kind="ExternalInput").ap()
    Wq_d = nc.dram_tensor("Wq", [H, H], f32, kind="ExternalInput").ap()
    bq_d = nc.dram_tensor("bq", [H], f32, kind="ExternalInput").ap()
    Wv_d = nc.dram_tensor("Wv", [H, H], f32, kind="ExternalInput").ap()
    bv_d = nc.dram_tensor("bv", [H], f32, kind="ExternalInput").ap()
    Wo_d = nc.dram_tensor("Wo", [H, 1], f32, kind="ExternalInput").ap()
    bo_d = nc.dram_tensor("bo", [1], f32, kind="ExternalInput").ap()  # unused (cancels)
    out_d = nc.dram_tensor("out", [b_per_core, H], f32, kind="ExternalOutput").ap()

    # value viewed so one [b, blk] slice is [128p, 2tp, 2j, 512h] with
    # s = blk*512 + tp*256 + p*2 + j  (s-pairs per partition -> 4KB DRAM runs)
    val_v = val_d.rearrange("b (blk tp p j) h -> b blk p tp j h", blk=nblk, tp=2, p=128, j=2)
    # chunked rows (matches xbar out layout h = 128k + p): W_sb[p, c, o] = W[c*128 + p, o]
    Wv_v = Wv_d.rearrange("(c p) o -> p c o", p=128)
    Wq_v = Wq_d.rearrange("(c p) o -> p c o", p=128)
    Wo_nat_v = Wo_d.rearrange("(r c) one -> r (c one)", c=128)  # [4, 128]
    bq_v = bq_d.rearrange("(r c) -> r c", c=128)                # [4, 128]
    bv_v = bv_d.rearrange("(r c) -> r c", c=128)

    with tile.TileContext(nc) as tc:
        with (
            tc.tile_pool(name="weights", bufs=1) as wpool,
            tc.tile_pool(name="vnat", bufs=2 * nblk) as vpool,
        ):
            # persistent SBUF residents
            Wv_sb = wpool.tile([128, HC, H], bf16)
            Wq_sb = wpool.tile([128, HC, H], bf16)
            Wo_sb = wpool.tile([128, HC], bf16)
            qcombT = wpool.tile([128, HC, b_per_core], f32)
            ones_bf = wpool.tile([1, 128], bf16)
            ones_f = wpool.tile([1, 1], f32)

            nc.gpsimd.dma_start(out=Wv_sb[:], in_=Wv_v)
            nc.gpsimd.dma_start(out=Wq_sb[:], in_=Wq_v)
            nc.gpsimd.memset(ones_bf[:], 1.0)
            nc.gpsimd.memset(ones_f[:], 1.0)

            with (
                tc.tile_pool(name="setup", bufs=1) as spool,
                tc.tile_pool(name="setup_psum", bufs=2, space="PSUM") as spsum,
            ):
                id4 = spool.tile([4, 4], bf16)
                make_identity(nc, id4[:])
                id4f = spool.tile([4, 4], f32)
                make_identity(nc, id4f[:])

                q_nat = spool.tile([b_per_core, H], bf16)
                nc.gpsimd.dma_start(out=q_nat[:], in_=q_d)

                # Wo^T: load [4,128] bf16, PE-transpose -> [128, 4]
                Wo_nat = spool.tile([4, 128], bf16)
                nc.gpsimd.dma_start(out=Wo_nat[:], in_=Wo_nat_v)

                # batch-0 value loads right after the small setup loads so the
                # SDMA engines start the big transfers early
                vts0 = []
                for blk in range(nblk):
                    vt = vpool.tile([128, 2, 2, H], bf16, tag="vnat")
                    nc.gpsimd.dma_start(out=vt[:], in_=val_v[0, blk])
                    vts0.append(vt)
                ps_wo = spsum.tile([128, HC], f32, tag="ps_s")
                nc.tensor.matmul(ps_wo[:], Wo_nat[:], id4[:], start=True, stop=True)
                nc.vector.tensor_copy(Wo_sb[:], ps_wo[:])

                # (bq+bv)^T: load [4,128] f32, add, PE-transpose -> [128, 4]
                bq_s = spool.tile([4, 128], f32)
                bv_s = spool.tile([4, 128], f32)
                nc.scalar.dma_start(out=bq_s[:], in_=bq_v)
                nc.scalar.dma_start(out=bv_s[:], in_=bv_v)
                bqv = spool.tile([4, 128], f32)
                nc.vector.tensor_add(bqv[:], bq_s[:], bv_s[:])
                ps_b = spsum.tile([128, HC], f32, tag="ps_s")
                nc.tensor.matmul(ps_b[:], bqv[:], id4f[:], start=True, stop=True)
                bqvT = spool.tile([128, HC], f32)
                nc.vector.tensor_copy(bqvT[:], ps_b[:])

                # q^T chunks: [128, B] per hic
                qT = spool.tile([128, HC, b_per_core], bf16)
                for hic in range(HC):
                    ps_q = spsum.tile([128, b_per_core], f32, tag="ps_s")
                    nc.tensor.matmul(
                        ps_q[:], q_nat[0:b_per_core, 128 * hic:128 * (hic + 1)],
                        id4[0:b_per_core, 0:b_per_core], start=True, stop=True,
                    )
                    nc.vector.tensor_copy(qT[:, hic, :], ps_q[:])

                # qcombT[ho, b] = (q[b] @ Wq)[ho] + bq[ho] + bv[ho]
                for hoc in range(HC):
                    ps_qp = spsum.tile([128, b_per_core], f32, tag="ps_s")
                    for hic in range(HC):
                        nc.tensor.matmul(
                            ps_qp[:], Wq_sb[:, hic, 128 * hoc:128 * (hoc + 1)],
                            qT[:, hic, :], start=(hic == 0), stop=(hic == HC - 1),
                        )
                    nc.scalar.activation(
                        qcombT[:, hoc, :], ps_qp[:], AF.Identity,
                        bias=bqvT[:, hoc:hoc + 1],
                    )

            with (
                tc.tile_pool(name="vt", bufs=8) as tpool,
                tc.tile_pool(name="ht", bufs=3) as hpool,
                tc.tile_pool(name="scrow", bufs=2) as scpool,
                tc.tile_pool(name="esc", bufs=2) as epool,
                tc.tile_pool(name="small", bufs=8) as smpool,
                tc.tile_pool(name="psum_h", bufs=4, space="PSUM") as psh,
                tc.tile_pool(name="psum_sc", bufs=2, space="PSUM") as pss,
                tc.tile_pool(name="psum_e", bufs=1, space="PSUM") as pse_pool,
                tc.tile_pool(name="psum_ctx", bufs=1, space="PSUM") as psc,
            ):
                def emit_tail(b, vts, esc, rec):
                    """context matmuls + normalize + store."""
                    ps_ctx = psc.tile([1, H], f32, tag="ctx")
                    for t in range(nt):
                        nc.tensor.matmul(
                            ps_ctx[:], esc[:, t:t + 1],
                            vts[t // 4][:, (t % 4) // 2, t % 2, :],
                            start=(t == 0), stop=(t == nt - 1),
                        )
                    outrow = smpool.tile([1, H], f32, tag="outrow")
                    nc.vector.tensor_scalar_mul(outrow[:], ps_ctx[:], rec[:])
                    nc.gpsimd.dma_start(out=out_d[b:b + 1, :], in_=outrow[:])

                pending = None
                for b in range(b_per_core):
                    if b == 0:
                        vts = vts0
                    else:
                        vts = []
                        for blk in range(nblk):
                            vt = vpool.tile([128, 2, 2, H], bf16, tag="vnat")
                            nc.gpsimd.dma_start(out=vt[:], in_=val_v[b, blk])
                            vts.append(vt)

                    scrow = scpool.tile([1, seq], f32, tag="scrow")
                    scT = scpool.tile([128, nt], f32, tag="scT")

                    for blk in range(nblk):
                        # one contiguous xbar op per blk:
                        # vT[p, jj, s2] with jj = tp*8 + j*4 + k, h = 128k + p
                        vT = tpool.tile([128, 4 * HC, 128], bf16, tag="vt")
                        nc.sync.dma_start_transpose(out=vT[:], in_=vts[blk][:])
                        # per-k moving view [128, 2tp, 2j, 128s2] (free = 512)
                        vTv = vT[:].rearrange("p (tp j k) s -> p k tp j s", k=HC, j=2)
                        hT = hpool.tile([128, HC, H], bf16, tag="ht")
                        for hoc in range(HC):
                            ps = psh.tile([128, H], f32, tag="ph")
                            for k in range(HC):
                                nc.tensor.matmul(
                                    ps[:], Wv_sb[:, k, 128 * hoc:128 * (hoc + 1)],
                                    vTv[:, k], start=(k == 0), stop=(k == HC - 1),
                                )
                            nc.scalar.activation(
                                hT[:, hoc, :], ps[:], AF.Tanh,
                                bias=qcombT[:, hoc, b:b + 1],
                            )
                        ps_sc = pss.tile([1, H], f32, tag="sc")
                        for hoc in range(HC):
                            nc.tensor.matmul(
                                ps_sc[:], Wo_sb[:, hoc:hoc + 1], hT[:, hoc, :],
                                start=(hoc == 0), stop=(hoc == HC - 1),
                            )
                        nc.vector.tensor_copy(
                            scrow[0:1, 512 * blk:512 * (blk + 1)], ps_sc[:],
                        )
                        for c in range(4):
                            t = 4 * blk + c
                            pse = pse_pool.tile([128, 1], f32, tag="pse")
                            nc.tensor.matmul(
                                pse[:], scrow[0:1, 128 * t:128 * (t + 1)],
                                ones_f[:], start=True, stop=True,
                            )
                            nc.vector.tensor_copy(scT[:, t:t + 1], pse[:])
                        if pending is not None and blk == min(1, nblk - 1):
                            emit_tail(*pending)
                            pending = None

                    esc = epool.tile([128, nt], bf16, tag="esc")
                    sumcol = smpool.tile([128, 1], f32, tag="sumcol")
                    nc.scalar.activation(
                        esc[:], scT[:], AF.Exp, accum_out=sumcol[:],
                    )
                    total = smpool.tile([128, 1], f32, tag="total")
                    nc.gpsimd.partition_all_reduce(
                        total[:], sumcol[:], 128, ReduceOp.add,
                    )
                    rec = smpool.tile([1, 1], f32, tag="rec")
                    nc.vector.reciprocal(rec[:], total[0:1, :])
                    pending = (b, vts, esc, rec)

                emit_tail(*pending)

    nc.compile()
    return nc


def kernel(**inputs):
    from concourse.bass_utils import run_bass_kernel_spmd

    key = "full"
    if key not in _cache:
        _cache[key] = build_nc()
    nc = _cache[key]

    query = np.asarray(inputs["query"], dtype=np.float32)   # [1, 32, 512]
    value = np.asarray(inputs["value"], dtype=np.float32)   # [32, 4096, 512]
    Wq = np.asarray(inputs["Wq"], dtype=np.float32)
    bq = np.asarray(inputs["bq"], dtype=np.float32)
    Wv = np.asarray(inputs["Wv"], dtype=np.float32)
    bv = np.asarray(inputs["bv"], dtype=np.float32)
    Wo = np.asarray(inputs["Wo"], dtype=np.float32)
    bo = np.asarray(inputs["bo"], dtype=np.float32)

    in_maps = []
    for i in range(N_CORES):
        sl = slice(B * i, B * (i + 1))
        in_maps.append({
            "value": np.ascontiguousarray(value[sl]),
            "query": np.ascontiguousarray(query[0, sl, :]),
            "Wq": Wq, "bq": bq, "Wv": Wv, "bv": bv, "Wo": Wo, "bo": bo,
        })

    res = run_bass_kernel_spmd(nc, in_maps, core_ids=list(range(N_CORES)))
    out = np.concatenate([res.results[i]["out"] for i in range(N_CORES)], axis=0)
    return out[:, None, :].astype(np.float32)  # [32, 1, 512]


# revision 23
# speedup vs baseline: 1.0103x; 1.0103x over previous
"""AdditiveAttention (Bahdanau) distributed Bass kernel for 8 TRN2 NeuronCores.

Computation (per batch b):
    qc[b,:]   = query[b] @ Wq + bq + bv                       # [512]
    z[b,s,:]  = value[b,s] @ Wv + qc[b]                       # pre-tanh
    score     = tanh(z) @ Wo          (+bo dropped: cancels in softmax)
    align     = softmax(score)        (no max-sub: |score| <= ~23, exp fits f32)
    out[b,:]  = align @ value[b]

Sharding: data-parallel over batch, 4 batches per core, weights replicated.

Per-core dataflow (B=4 batches, SEQ=4096, H=512), hidden-TRANSPOSED layout:
  - value loaded HBM->SBUF with f32->bf16 cast DMA (SWDGE), natural layout
    v_nat[128s, 4t, 512h] per 512-seq block.
  - xbar DMA-transpose (HWDGE) per s-tile: [128s,512h] -> vT[128p,4k,128s]
    with h = 128k + p (verified on HW).
  - main mm per (blk, hoc): psum_hT[128ho, 512s] = sum_k Wv[:,k,hoc].T @ vT[:,k,:]
    (Wv chunk stationary, vT moving, N=512).
  - tanh on ACT with per-partition bias qcombT[ho] -> hT bf16 (query proj free).
  - score per blk: psum[1,512] = sum_hoc Wo[:,hoc].T @ hT[:,hoc,:]; DVE copy
    into scrow[1,4096].
  - one Exp per batch on ACT [1,4096] with accum_out -> esc_row bf16 + total;
    DVE reciprocal.
  - escT via 32 tiny PE transpose-matmuls (esc row chunk [1,128] stationary,
    ones rhs) -> psum [128,1] -> DVE copy -> escT[128,32] bf16.
  - context: psum[1,512] += escT[:,t].T @ v_nat (32 accumulating mms);
    DVE tensor_scalar_mul by 1/total -> out row.
  - batch tail (escT+ctx+out) is emitted 2 blocks into the NEXT batch so the
    PE stream never drains at batch boundaries (HAM stays warm).
"""

import numpy as np

N_CORES = 8
BATCH_TOTAL = 32
B = BATCH_TOTAL // N_CORES  # batches per core
SEQ = 4096
H = 512
HC = H // 128  # 4 hidden chunks

_cache = {}


def build_nc(b_per_core=B, seq=SEQ):
    import concourse.bass as bass
    import concourse.mybir as mybir
    import concourse.tile as tile
    from concourse import bacc
    from concourse.masks import make_identity
    from concourse.bass_isa import ReduceOp

    f32 = mybir.dt.float32
    bf16 = mybir.dt.bfloat16
    AF = mybir.ActivationFunctionType
    ALU = mybir.AluOpType
    AX = mybir.AxisListType

    nblk = seq // 512   # 512-seq blocks
    nt = seq // 128     # 128-seq tiles

    nc = bacc.Bacc("TRN2", target_bir_lowering=False, debug=False)

    val_d = nc.dram_tensor("value", [b_per_core, seq, H], f32, kind="ExternalInput").ap()
    q_d = nc.dram_tensor("query", [b_per_core, H], f32, kind="ExternalInput").ap()
    Wq_d = nc.dram_tensor("Wq", [H, H], f32, kind="ExternalInput").ap()
    bq_d = nc.dram_tensor("bq", [H], f32, kind="ExternalInput").ap()
    Wv_d = nc.dram_tensor("Wv", [H, H], f32, kind="ExternalInput").ap()
    bv_d = nc.dram_tensor("bv", [H], f32, kind="ExternalInput").ap()
    Wo_d = nc.dram_tensor("Wo", [H, 1], f32, kind="ExternalInput").ap()
    bo_d = nc.dram_tensor("bo", [1], f32, kind="ExternalInput").ap()  # unused (cancels)
    out_d = nc.dram_tensor("out", [b_per_core, H], f32, kind="ExternalOutput").ap()

    # value viewed so one [b, blk] slice is [128p, 2tp, 2j, 512h] with
    # s = blk*512 + tp*256 + p*2 + j  (s-pairs per partition -> 4KB DRAM runs)
    val_v = val_d.rearrange("b (blk tp p j) h -> b blk p tp j h", blk=nblk, tp=2, p=128, j=2)
    # chunked rows (matches xbar out layout h = 128k + p): W_sb[p, c, o] = W[c*128 + p, o]
    Wv_v = Wv_d.rearrange("(c p) o -> p c o", p=128)
    Wq_v = Wq_d.rearrange("(c p) o -> p c o", p=128)
    Wo_nat_v = Wo_d.rearrange("(r c) one -> r (c one)", c=128)  # [4, 128]
    bq_v = bq_d.rearrange("(r c) -> r c", c=128)                # [4, 128]
    bv_v = bv_d.rearrange("(r c) -> r c", c=128)

    with tile.TileContext(nc) as tc:
        with (
            tc.tile_pool(name="weights", bufs=1) as wpool,
            tc.tile_pool(name="vnat", bufs=2 * nblk) as vpool,
        ):
            # persistent SBUF residents
            Wv_sb = wpool.tile([128, HC, H], bf16)
            Wq_sb = wpool.tile([128, HC, H], bf16)
            Wo_sb = wpool.tile([128, HC], bf16)
            qcombT = wpool.tile([128, HC, b_per_core], f32)
            ones_bf = wpool.tile([1, 128], bf16)
            ones_f = wpool.tile([1, 1], f32)

            nc.gpsimd.dma_start(out=Wv_sb[:], in_=Wv_v)
            nc.gpsimd.dma_start(out=Wq_sb[:], in_=Wq_v)
            nc.gpsimd.memset(ones_bf[:], 1.0)
            nc.gpsimd.memset(ones_f[:], 1.0)

            with (
                tc.tile_pool(name="setup", bufs=1) as spool,
                tc.tile_pool(name="setup_psum", bufs=2, space="PSUM") as spsum,
            ):
                id4 = spool.tile([4, 4], bf16)
                make_identity(nc, id4[:])
                id4f = spool.tile([4, 4], f32)
                make_identity(nc, id4f[:])

                q_nat = spool.tile([b_per_core, H], bf16)
                nc.gpsimd.dma_start(out=q_nat[:], in_=q_d)

                # Wo^T: load [4,128] bf16, PE-transpose -> [128, 4]
                Wo_nat = spool.tile([4, 128], bf16)
                nc.gpsimd.dma_start(out=Wo_nat[:], in_=Wo_nat_v)

                # batch-0 value loads right after the small setup loads so the
                # SDMA engines start the big transfers early
                vts0 = []
                for blk in range(nblk):
                    vt = vpool.tile([128, 2, 2, H], bf16, tag="vnat")
                    nc.gpsimd.dma_start(out=vt[:], in_=val_v[0, blk])
                    vts0.append(vt)
                ps_wo = spsum.tile([128, HC], f32, tag="ps_s")
                nc.tensor.matmul(ps_wo[:], Wo_nat[:], id4[:], start=True, stop=True)
                nc.vector.tensor_copy(Wo_sb[:], ps_wo[:])

                # (bq+bv)^T: load [4,128] f32, add, PE-transpose -> [128, 4]
                bq_s = spool.tile([4, 128], f32)
                bv_s = spool.tile([4, 128], f32)
                nc.scalar.dma_start(out=bq_s[:], in_=bq_v)
                nc.scalar.dma_start(out=bv_s[:], in_=bv_v)
                bqv = spool.tile([4, 128], f32)
                nc.vector.tensor_add(bqv[:], bq_s[:], bv_s[:])
                ps_b = spsum.tile([128, HC], f32, tag="ps_s")
                nc.tensor.matmul(ps_b[:], bqv[:], id4f[:], start=True, stop=True)
                bqvT = spool.tile([128, HC], f32)
                nc.vector.tensor_copy(bqvT[:], ps_b[:])

                # q^T chunks: [128, B] per hic
                qT = spool.tile([128, HC, b_per_core], bf16)
                for hic in range(HC):
                    ps_q = spsum.tile([128, b_per_core], f32, tag="ps_s")
                    nc.tensor.matmul(
                        ps_q[:], q_nat[0:b_per_core, 128 * hic:128 * (hic + 1)],
                        id4[0:b_per_core, 0:b_per_core], start=True, stop=True,
                    )
                    nc.vector.tensor_copy(qT[:, hic, :], ps_q[:])

                # qcombT[ho, b] = (q[b] @ Wq)[ho] + bq[ho] + bv[ho]
                for hoc in range(HC):
                    ps_qp = spsum.tile([128, b_per_core], f32, tag="ps_s")
                    for hic in range(HC):
                        nc.tensor.matmul(
                            ps_qp[:], Wq_sb[:, hic, 128 * hoc:128 * (hoc + 1)],
                            qT[:, hic, :], start=(hic == 0), stop=(hic == HC - 1),
                        )
                    nc.scalar.activation(
                        qcombT[:, hoc, :], ps_qp[:], AF.Identity,
                        bias=bqvT[:, hoc:hoc + 1],
                    )

            with (
                tc.tile_pool(name="vt", bufs=8) as tpool,
                tc.tile_pool(name="ht", bufs=3) as hpool,
                tc.tile_pool(name="scrow", bufs=2) as scpool,
                tc.tile_pool(name="esc", bufs=2) as epool,
                tc.tile_pool(name="small", bufs=8) as smpool,
                tc.tile_pool(name="psum_h", bufs=4, space="PSUM") as psh,
                tc.tile_pool(name="psum_sc", bufs=2, space="PSUM") as pss,
                tc.tile_pool(name="psum_e", bufs=1, space="PSUM") as pse_pool,
                tc.tile_pool(name="psum_ctx", bufs=1, space="PSUM") as psc,
            ):
                def emit_tail(b, vts, esc, rec):
                    """context matmuls + normalize + store."""
                    ps_ctx = psc.tile([1, H], f32, tag="ctx")
                    for t in range(nt):
                        nc.tensor.matmul(
                            ps_ctx[:], esc[:, t:t + 1],
                            vts[t // 4][:, (t % 4) // 2, t % 2, :],
                            start=(t == 0), stop=(t == nt - 1),
                        )
                    outrow = smpool.tile([1, H], f32, tag="outrow")
                    nc.vector.tensor_scalar_mul(outrow[:], ps_ctx[:], rec[:])
                    nc.gpsimd.dma_start(out=out_d[b:b + 1, :], in_=outrow[:])

                pending = None
                for b in range(b_per_core):
                    if b == 0:
                        vts = vts0
                    else:
                        vts = []
                        for blk in range(nblk):
                            vt = vpool.tile([128, 2, 2, H], bf16, tag="vnat")
                            nc.gpsimd.dma_start(out=vt[:], in_=val_v[b, blk])
                            vts.append(vt)

                    scrow = scpool.tile([1, seq], f32, tag="scrow")
                    scT = scpool.tile([128, nt], f32, tag="scT")

                    for blk in range(nblk):
                        # one contiguous xbar op per blk:
                        # vT[p, jj, s2] with jj = tp*8 + j*4 + k, h = 128k + p
                        vT = tpool.tile([128, 4 * HC, 128], bf16, tag="vt")
                        nc.sync.dma_start_transpose(out=vT[:], in_=vts[blk][:])
                        # per-k moving view [128, 2tp, 2j, 128s2] (free = 512)
                        vTv = vT[:].rearrange("p (tp j k) s -> p k tp j s", k=HC, j=2)
                        hT = hpool.tile([128, HC, H], bf16, tag="ht")
                        for hoc in range(HC):
                            ps = psh.tile([128, H], f32, tag="ph")
                            for k in range(HC):
                                nc.tensor.matmul(
                                    ps[:], Wv_sb[:, k, 128 * hoc:128 * (hoc + 1)],
                                    vTv[:, k], start=(k == 0), stop=(k == HC - 1),
                                )
                            nc.scalar.activation(
                                hT[:, hoc, :], ps[:], AF.Tanh,
                                bias=qcombT[:, hoc, b:b + 1],
                            )
                        ps_sc = pss.tile([1, H], f32, tag="sc")
                        for hoc in range(HC):
                            nc.tensor.matmul(
                                ps_sc[:], Wo_sb[:, hoc:hoc + 1], hT[:, hoc, :],
                                start=(hoc == 0), stop=(hoc == HC - 1),
                            )
                        nc.vector.tensor_copy(
                            scrow[0:1, 512 * blk:512 * (blk + 1)], ps_sc[:],
                        )
                        # transpose the PREVIOUS blk's scores (1-blk lag so the
                        # PE queue never waits on the DVE scrow copy)
                        for pb in ([blk - 1] if blk > 0 else []) + ([blk] if blk == nblk - 1 else []):
                            for c in range(4):
                                t = 4 * pb + c
                                pse = pse_pool.tile([128, 1], f32, tag="pse")
                                nc.tensor.matmul(
                                    pse[:], scrow[0:1, 128 * t:128 * (t + 1)],
                                    ones_f[:], start=True, stop=True,
                                )
                                nc.vector.tensor_copy(scT[:, t:t + 1], pse[:])
                        if pending is not None and blk == min(1, nblk - 1):
                            emit_tail(*pending)
                            pending = None

                    esc = epool.tile([128, nt], bf16, tag="esc")
                    sumcol = smpool.tile([128, 1], f32, tag="sumcol")
                    nc.scalar.activation(
                        esc[:], scT[:], AF.Exp, accum_out=sumcol[:],
                    )
                    total = smpool.tile([128, 1], f32, tag="total")
                    nc.gpsimd.partition_all_reduce(
                        total[:], sumcol[:], 128, ReduceOp.add,
                    )
                    rec = smpool.tile([1, 1], f32, tag="rec")
                    nc.vector.reciprocal(rec[:], total[0:1, :])
                    pending = (b, vts, esc, rec)

                emit_tail(*pending)

    nc.compile()
    return nc


def kernel(**inputs):
    from concourse.bass_utils import run_bass_kernel_spmd

    key = "full"
    if key not in _cache:
        _cache[key] = build_nc()
    nc = _cache[key]

    query = np.asarray(inputs["query"], dtype=np.float32)   # [1, 32, 512]
    value = np.asarray(inputs["value"], dtype=np.float32)   # [32, 4096, 512]
    Wq = np.asarray(inputs["Wq"], dtype=np.float32)
    bq = np.asarray(inputs["bq"], dtype=np.float32)
    Wv = np.asarray(inputs["Wv"], dtype=np.float32)
    bv = np.asarray(inputs["bv"], dtype=np.float32)
    Wo = np.asarray(inputs["Wo"], dtype=np.float32)
    bo = np.asarray(inputs["bo"], dtype=np.float32)

    in_maps = []
    for i in range(N_CORES):
        sl = slice(B * i, B * (i + 1))
        in_maps.append({
            "value": np.ascontiguousarray(value[sl]),
            "query": np.ascontiguousarray(query[0, sl, :]),
            "Wq": Wq, "bq": bq, "Wv": Wv, "bv": bv, "Wo": Wo, "bo": bo,
        })

    res = run_bass_kernel_spmd(nc, in_maps, core_ids=list(range(N_CORES)))
    out = np.concatenate([res.results[i]["out"] for i in range(N_CORES)], axis=0)
    return out[:, None, :].astype(np.float32)  # [32, 1, 512]


# revision 25
# speedup vs baseline: 1.0348x; 1.0243x over previous
"""AdditiveAttention (Bahdanau) distributed Bass kernel for 8 TRN2 NeuronCores.

Computation (per batch b):
    qc[b,:]   = query[b] @ Wq + bq + bv                       # [512]
    z[b,s,:]  = value[b,s] @ Wv + qc[b]                       # pre-tanh
    score     = tanh(z) @ Wo          (+bo dropped: cancels in softmax)
    align     = softmax(score)        (no max-sub: |score| <= ~23, exp fits f32)
    out[b,:]  = align @ value[b]

Sharding: data-parallel over batch, 4 batches per core, weights replicated.

Per-core dataflow (B=4 batches, SEQ=4096, H=512), hidden-TRANSPOSED layout:
  - value loaded HBM->SBUF with f32->bf16 cast DMA (SWDGE), natural layout
    v_nat[128s, 4t, 512h] per 512-seq block.
  - xbar DMA-transpose (HWDGE) per s-tile: [128s,512h] -> vT[128p,4k,128s]
    with h = 128k + p (verified on HW).
  - main mm per (blk, hoc): psum_hT[128ho, 512s] = sum_k Wv[:,k,hoc].T @ vT[:,k,:]
    (Wv chunk stationary, vT moving, N=512).
  - tanh on ACT with per-partition bias qcombT[ho] -> hT bf16 (query proj free).
  - score per blk: psum[1,512] = sum_hoc Wo[:,hoc].T @ hT[:,hoc,:]; DVE copy
    into scrow[1,4096].
  - one Exp per batch on ACT [1,4096] with accum_out -> esc_row bf16 + total;
    DVE reciprocal.
  - escT via 32 tiny PE transpose-matmuls (esc row chunk [1,128] stationary,
    ones rhs) -> psum [128,1] -> DVE copy -> escT[128,32] bf16.
  - context: psum[1,512] += escT[:,t].T @ v_nat (32 accumulating mms);
    DVE tensor_scalar_mul by 1/total -> out row.
  - batch tail (escT+ctx+out) is emitted 2 blocks into the NEXT batch so the
    PE stream never drains at batch boundaries (HAM stays warm).
"""

import numpy as np

N_CORES = 8
BATCH_TOTAL = 32
B = BATCH_TOTAL // N_CORES  # batches per core
SEQ = 4096
H = 512
HC = H // 128  # 4 hidden chunks

_cache = {}


def build_nc(b_per_core=B, seq=SEQ):
    import concourse.bass as bass
    import concourse.mybir as mybir
    import concourse.tile as tile
    from concourse import bacc
    from concourse.masks import make_identity
    from concourse.bass_isa import ReduceOp

    f32 = mybir.dt.float32
    bf16 = mybir.dt.bfloat16
    AF = mybir.ActivationFunctionType
    ALU = mybir.AluOpType
    AX = mybir.AxisListType

    nblk = seq // 512   # 512-seq blocks
    nt = seq // 128     # 128-seq tiles

    nc = bacc.Bacc("TRN2", target_bir_lowering=False, debug=False)

    val_d = nc.dram_tensor("value", [b_per_core, seq, H], f32, kind="ExternalInput").ap()
    q_d = nc.dram_tensor("query", [b_per_core, H], f32, kind="ExternalInput").ap()
    Wq_d = nc.dram_tensor("Wq", [H, H], f32, kind="ExternalInput").ap()
    bq_d = nc.dram_tensor("bq", [H], f32, kind="ExternalInput").ap()
    Wv_d = nc.dram_tensor("Wv", [H, H], f32, kind="ExternalInput").ap()
    bv_d = nc.dram_tensor("bv", [H], f32, kind="ExternalInput").ap()
    Wo_d = nc.dram_tensor("Wo", [H, 1], f32, kind="ExternalInput").ap()
    bo_d = nc.dram_tensor("bo", [1], f32, kind="ExternalInput").ap()  # unused (cancels)
    out_d = nc.dram_tensor("out", [b_per_core, H], f32, kind="ExternalOutput").ap()

    # value viewed so one [b, blk] slice is [128p, 4j, 512h] with
    # s = blk*512 + p*4 + j  (4 consecutive s-rows per partition -> 8KB DRAM runs)
    val_v = val_d.rearrange("b (blk p j) h -> b blk p j h", blk=nblk, p=128, j=4)
    # chunked rows (matches xbar out layout h = 128k + p): W_sb[p, c, o] = W[c*128 + p, o]
    Wv_v = Wv_d.rearrange("(c p) o -> p c o", p=128)
    Wq_v = Wq_d.rearrange("(c p) o -> p c o", p=128)
    Wo_nat_v = Wo_d.rearrange("(r c) one -> r (c one)", c=128)  # [4, 128]
    bq_v = bq_d.rearrange("(r c) -> r c", c=128)                # [4, 128]
    bv_v = bv_d.rearrange("(r c) -> r c", c=128)

    with tile.TileContext(nc) as tc:
        with (
            tc.tile_pool(name="weights", bufs=1) as wpool,
            tc.tile_pool(name="vnat", bufs=2 * nblk) as vpool,
        ):
            # persistent SBUF residents
            Wv_sb = wpool.tile([128, HC, H], bf16)
            Wq_sb = wpool.tile([128, HC, H], bf16)
            Wo_sb = wpool.tile([128, HC], bf16)
            qcombT = wpool.tile([128, HC, b_per_core], f32)
            ones_bf = wpool.tile([1, 128], bf16)
            ones_f = wpool.tile([1, 1], f32)

            nc.gpsimd.dma_start(out=Wv_sb[:], in_=Wv_v)
            nc.gpsimd.dma_start(out=Wq_sb[:], in_=Wq_v)
            nc.gpsimd.memset(ones_bf[:], 1.0)
            nc.gpsimd.memset(ones_f[:], 1.0)

            with (
                tc.tile_pool(name="setup", bufs=1) as spool,
                tc.tile_pool(name="setup_psum", bufs=2, space="PSUM") as spsum,
            ):
                id4 = spool.tile([4, 4], bf16)
                make_identity(nc, id4[:])
                id4f = spool.tile([4, 4], f32)
                make_identity(nc, id4f[:])

                q_nat = spool.tile([b_per_core, H], bf16)
                nc.gpsimd.dma_start(out=q_nat[:], in_=q_d)

                # Wo^T: load [4,128] bf16, PE-transpose -> [128, 4]
                Wo_nat = spool.tile([4, 128], bf16)
                nc.gpsimd.dma_start(out=Wo_nat[:], in_=Wo_nat_v)

                # batch-0 value loads right after the small setup loads so the
                # SDMA engines start the big transfers early
                vts0 = []
                for blk in range(nblk):
                    vt = vpool.tile([128, 4, H], bf16, tag="vnat")
                    nc.gpsimd.dma_start(out=vt[:], in_=val_v[0, blk])
                    vts0.append(vt)
                ps_wo = spsum.tile([128, HC], f32, tag="ps_s")
                nc.tensor.matmul(ps_wo[:], Wo_nat[:], id4[:], start=True, stop=True)
                nc.vector.tensor_copy(Wo_sb[:], ps_wo[:])

                # (bq+bv)^T: load [4,128] f32, add, PE-transpose -> [128, 4]
                bq_s = spool.tile([4, 128], f32)
                bv_s = spool.tile([4, 128], f32)
                nc.scalar.dma_start(out=bq_s[:], in_=bq_v)
                nc.scalar.dma_start(out=bv_s[:], in_=bv_v)
                bqv = spool.tile([4, 128], f32)
                nc.vector.tensor_add(bqv[:], bq_s[:], bv_s[:])
                ps_b = spsum.tile([128, HC], f32, tag="ps_s")
                nc.tensor.matmul(ps_b[:], bqv[:], id4f[:], start=True, stop=True)
                bqvT = spool.tile([128, HC], f32)
                nc.vector.tensor_copy(bqvT[:], ps_b[:])

                # q^T chunks: [128, B] per hic
                qT = spool.tile([128, HC, b_per_core], bf16)
                for hic in range(HC):
                    ps_q = spsum.tile([128, b_per_core], f32, tag="ps_s")
                    nc.tensor.matmul(
                        ps_q[:], q_nat[0:b_per_core, 128 * hic:128 * (hic + 1)],
                        id4[0:b_per_core, 0:b_per_core], start=True, stop=True,
                    )
                    nc.vector.tensor_copy(qT[:, hic, :], ps_q[:])

                # qcombT[ho, b] = (q[b] @ Wq)[ho] + bq[ho] + bv[ho]
                for hoc in range(HC):
                    ps_qp = spsum.tile([128, b_per_core], f32, tag="ps_s")
                    for hic in range(HC):
                        nc.tensor.matmul(
                            ps_qp[:], Wq_sb[:, hic, 128 * hoc:128 * (hoc + 1)],
                            qT[:, hic, :], start=(hic == 0), stop=(hic == HC - 1),
                        )
                    nc.scalar.activation(
                        qcombT[:, hoc, :], ps_qp[:], AF.Identity,
                        bias=bqvT[:, hoc:hoc + 1],
                    )

            with (
                tc.tile_pool(name="vt", bufs=8) as tpool,
                tc.tile_pool(name="ht", bufs=3) as hpool,
                tc.tile_pool(name="scrow", bufs=2) as scpool,
                tc.tile_pool(name="esc", bufs=2) as epool,
                tc.tile_pool(name="small", bufs=8) as smpool,
                tc.tile_pool(name="psum_h", bufs=4, space="PSUM") as psh,
                tc.tile_pool(name="psum_sc", bufs=1, space="PSUM") as pss,
                tc.tile_pool(name="psum_e", bufs=2, space="PSUM") as pse_pool,
                tc.tile_pool(name="psum_ctx", bufs=1, space="PSUM") as psc,
            ):
                def emit_tail(b, vts, scrow, scT):
                    """last-blk score transposes, softmax, context, store."""
                    for c in range(4):
                        t = 4 * (nblk - 1) + c
                        pse = pse_pool.tile([128, 1], f32, tag="pse")
                        nc.tensor.matmul(
                            pse[:], scrow[0:1, 128 * t:128 * (t + 1)],
                            ones_f[:], start=True, stop=True,
                        )
                        nc.vector.tensor_copy(scT[:, t:t + 1], pse[:])
                    esc = epool.tile([128, nt], bf16, tag="esc")
                    sumcol = smpool.tile([128, 1], f32, tag="sumcol")
                    nc.scalar.activation(
                        esc[:], scT[:], AF.Exp, accum_out=sumcol[:],
                    )
                    total = smpool.tile([128, 1], f32, tag="total")
                    nc.gpsimd.partition_all_reduce(
                        total[:], sumcol[:], 128, ReduceOp.add,
                    )
                    rec = smpool.tile([1, 1], f32, tag="rec")
                    nc.vector.reciprocal(rec[:], total[0:1, :])
                    ps_ctx = psc.tile([1, H], f32, tag="ctx")
                    for t in range(nt):
                        nc.tensor.matmul(
                            ps_ctx[:], esc[:, t:t + 1],
                            vts[t // 4][:, t % 4, :],
                            start=(t == 0), stop=(t == nt - 1),
                        )
                    outrow = smpool.tile([1, H], f32, tag="outrow")
                    nc.vector.tensor_scalar_mul(outrow[:], ps_ctx[:], rec[:])
                    nc.gpsimd.dma_start(out=out_d[b:b + 1, :], in_=outrow[:])

                pending = None
                for b in range(b_per_core):
                    if b == 0:
                        vts = vts0
                    else:
                        vts = []
                        for blk in range(nblk):
                            vt = vpool.tile([128, 4, H], bf16, tag="vnat")
                            nc.gpsimd.dma_start(out=vt[:], in_=val_v[b, blk])
                            vts.append(vt)

                    scrow = scpool.tile([1, seq], f32, tag="scrow")
                    scT = scpool.tile([128, nt], f32, tag="scT")

                    for blk in range(nblk):
                        # one contiguous xbar op per blk:
                        # vT[p, jj, s2] with jj = j*4 + k, h = 128k + p
                        vT = tpool.tile([128, 4 * HC, 128], bf16, tag="vt")
                        nc.sync.dma_start_transpose(out=vT[:], in_=vts[blk][:])
                        # per-k moving view [128, 4j, 128s2] (free = 512)
                        vTv = vT[:].rearrange("p (j k) s -> p k j s", k=HC)
                        hT = hpool.tile([128, HC, H], bf16, tag="ht")
                        for hoc in range(HC):
                            ps = psh.tile([128, H], f32, tag="ph")
                            for k in range(HC):
                                nc.tensor.matmul(
                                    ps[:], Wv_sb[:, k, 128 * hoc:128 * (hoc + 1)],
                                    vTv[:, k], start=(k == 0), stop=(k == HC - 1),
                                )
                            nc.scalar.activation(
                                hT[:, hoc, :], ps[:], AF.Tanh,
                                bias=qcombT[:, hoc, b:b + 1],
                            )
                        ps_sc = pss.tile([1, H], f32, tag="sc")
                        for hoc in range(HC):
                            nc.tensor.matmul(
                                ps_sc[:], Wo_sb[:, hoc:hoc + 1], hT[:, hoc, :],
                                start=(hoc == 0), stop=(hoc == HC - 1),
                            )
                        nc.vector.tensor_copy(
                            scrow[0:1, 512 * blk:512 * (blk + 1)], ps_sc[:],
                        )
                        # transpose the PREVIOUS blk's scores (1-blk lag so the
                        # PE queue never waits on the DVE scrow copy)
                        if blk > 0:
                            for c in range(4):
                                t = 4 * (blk - 1) + c
                                pse = pse_pool.tile([128, 1], f32, tag="pse")
                                nc.tensor.matmul(
                                    pse[:], scrow[0:1, 128 * t:128 * (t + 1)],
                                    ones_f[:], start=True, stop=True,
                                )
                                nc.vector.tensor_copy(scT[:, t:t + 1], pse[:])
                        if pending is not None and blk == min(1, nblk - 1):
                            emit_tail(*pending)
                            pending = None

                    pending = (b, vts, scrow, scT)

                emit_tail(*pending)

    nc.compile()
    return nc


def kernel(**inputs):
    from concourse.bass_utils import run_bass_kernel_spmd

    key = "full"
    if key not in _cache:
        _cache[key] = build_nc()
    nc = _cache[key]

    query = np.asarray(inputs["query"], dtype=np.float32)   # [1, 32, 512]
    value = np.asarray(inputs["value"], dtype=np.float32)   # [32, 4096, 512]
    Wq = np.asarray(inputs["Wq"], dtype=np.float32)
    bq = np.asarray(inputs["bq"], dtype=np.float32)
    Wv = np.asarray(inputs["Wv"], dtype=np.float32)
    bv = np.asarray(inputs["bv"], dtype=np.float32)
    Wo = np.asarray(inputs["Wo"], dtype=np.float32)
    bo = np.asarray(inputs["bo"], dtype=np.float32)

    in_maps = []
    for i in range(N_CORES):
        sl = slice(B * i, B * (i + 1))
        in_maps.append({
            "value": np.ascontiguousarray(value[sl]),
            "query": np.ascontiguousarray(query[0, sl, :]),
            "Wq": Wq, "bq": bq, "Wv": Wv, "bv": bv, "Wo": Wo, "bo": bo,
        })

    res = run_bass_kernel_spmd(nc, in_maps, core_ids=list(range(N_CORES)))
    out = np.concatenate([res.results[i]["out"] for i in range(N_CORES)], axis=0)
    return out[:, None, :].astype(np.float32)  # [32, 1, 512]
